# revision 7
# baseline (speedup 1.0000x reference)
"""BandSplitEncoder Trainium2 kernel, v3.

x[B,T,2048] -> 62 band RMSNorm+Linear[d->512] -> [B,T,62,512].
Data-parallel over the 2048 B*T tokens across 8 NeuronCores (256 each).

Per core:
  - x ships as a zero-padded packed fp16 image [128, 25*256] (25 chunks
    of 128 feature rows, bands slot-aligned, heavy bands first).
  - norm chain (all slices up front): DVE squares (f16 2x) -> per-chunk
    indicator matmuls reduce the partition dim -> ssq[band, tok] f32
    PSUM -> DVE reciprocal_approx_fast -> ACT sqrt -> inv[band, tok]
    f16 -> one SBUF->SBUF DMA flattens it to a single partition row ->
    13 partition-broadcast DMAs expand it to the [feat_row, tok] image
    -> DVE multiply (f16 2x) prescales x.
  - band matmuls (W carries gamma, sqrt(d) and the int8 scale, so PSUM
    holds final int8 code values): 4 packed bands per [128,2048] f32
    PSUM quad (4 banks, one pool bufs=2 = all 8 banks); small-K bands
    run concurrently via tile_position row groups.
  - evac = pure f32->int8 saturating copy per quad, split ACT/DVE by a
    measured-cost greedy; int8 out tiles DMA per packed-band range.
  - W ships compact [2080,512] f16 (chunk-consumption row order),
    scattered to the slot layout by 13 strided DMAs on the GpSimd
    queue; pad rows are never read.
Host folds gamma + sqrt(d) + per-band int8 scale into W, packs x, and
dequantizes + adds bias on the way out.
"""

import numpy as np

import concourse.bacc as bacc
import concourse.tile as tile
from concourse import mybir
from concourse.bass_utils import run_bass_kernel_spmd

# ---------------------------------------------------------------- problem dims
DIM_INPUTS = (4,) * 24 + (8,) * 12 + (24,) * 8 + (48,) * 8 + (96,) * 8 + (256,) * 2
N_BANDS = len(DIM_INPUTS)  # 62
F_TOTAL = sum(DIM_INPUTS)  # 2048
DIM = 512
B, T = 4, 512
BT = B * T
N_CORES = 8
TOK = BT // N_CORES  # 256
N_TILES = TOK // 128  # 2
ZQ = 6.0  # int8 quant margin in units of the per-band max W column norm
WGC_PAD = 2080  # wgc DRAM rows incl. slack so strided scatter APs stay in-bounds

OFFSETS = []
_off = 0
for _d in DIM_INPUTS:
    OFFSETS.append(_off)
    _off += _d

# ------------------------------------------------- packed chunk layout
CHUNKS = []
for b in (60, 61):  # d256: two full chunks each
    CHUNKS.append([(0, 128, b, 0)])
    CHUNKS.append([(0, 128, b, 128)])
for i in range(44, 52, 2):  # d48: two per chunk at slots 0/64
    CHUNKS.append([(0, 48, i, 0), (64, 48, i + 1, 0)])
for k in range(8):  # d96 at slot 0 + d24 at slot 96
    CHUNKS.append([(0, 96, 52 + k, 0), (96, 24, 36 + k, 0)])
for i in range(24, 36, 4):  # d8: four per chunk
    CHUNKS.append([(32 * j, 8, i + j, 0) for j in range(4)])
for i in range(0, 24, 4):  # d4: four per chunk
    CHUNKS.append([(32 * j, 4, i + j, 0) for j in range(4)])
N_CHUNKS = len(CHUNKS)  # 25
F_PACK = N_CHUNKS * 128  # 3200

PLACEMENT = [[] for _ in range(N_BANDS)]
PBANDS = []
for _c, _segs in enumerate(CHUNKS):
    for _slot, _n, _b, _soff in _segs:
        PLACEMENT[_b].append((_c, _slot, _n, OFFSETS[_b] + _soff))
        if _b not in PBANDS:
            PBANDS.append(_b)

ROW_MAP = np.full((F_PACK,), -1, dtype=np.int64)
for _b in range(N_BANDS):
    for _c, _slot, _n, _src in PLACEMENT[_b]:
        ROW_MAP[_c * 128 + _slot : _c * 128 + _slot + _n] = np.arange(_src, _src + _n)

WGC_ROWS = np.concatenate(
    [np.arange(OFFSETS[b] + soff, OFFSETS[b] + soff + n)
     for segs in CHUNKS for (slot, n, b, soff) in segs]
)
assert WGC_ROWS.shape[0] == F_TOTAL

# pipeline slices: (c0, c1, p0, p1) chunk/packed-band ranges
SLICES = [(0, 4, 0, 2), (4, 8, 2, 10), (8, 16, 10, 26),
          (16, 19, 26, 38), (19, 25, 38, 62)]

# out tiles: packed-band ranges (quad-aligned)
OUT_RANGES = [(0, 10), (10, 26), (26, 38), (38, 50), (50, 62)]

# WG scatter specs: (dst_part0, nrows, c0, nch, src_row0, src_chunk_stride)
WG_SCATTER = (
    [(0, 128, 0, 4, 0, 128),
     (0, 48, 4, 4, 512, 96), (64, 48, 4, 4, 560, 96),
     (0, 96, 8, 8, 896, 120), (96, 24, 8, 8, 992, 120)]
    + [(32 * s, 8, 16, 3, 1856 + 8 * s, 32) for s in range(4)]
    + [(32 * s, 4, 19, 6, 1952 + 4 * s, 16) for s in range(4)]
)

# inv-norm broadcast specs per slice:
# (part0, nparts, band_step_rows, first_band_rel, nch_grp, chunk_band_stride)
# dst = XSCb[part0:part0+nparts, (c0+j)*256 : ...] for j in 0..nch_grp
# src row (packed pos) = p0 + first_band_rel + j*chunk_band_stride
BCASTS = {
    0: [(0, 128, 0, [(0, 2, 0)])],  # handled specially (dup chunks)
}

# indicator column offsets per chunk
IND_OFF = []
_io = 0
for _c in range(N_CHUNKS):
    for (c0, c1, p0, p1) in SLICES:
        if c0 <= _c < c1:
            IND_OFF.append(_io)
            _io += p1 - p0
IND_COLS = _io  # 348

_CACHE = {}


def _build_program():
    nc = bacc.Bacc("TRN2", target_bir_lowering=False, debug=False, num_devices=N_CORES)
    f32 = mybir.dt.float32
    f16 = mybir.dt.float16
    i8 = mybir.dt.int8
    AF = mybir.ActivationFunctionType

    xt_ap = nc.dram_tensor("xt", [128, N_CHUNKS * TOK], f16, kind="ExternalInput").ap()
    wgc_ap = nc.dram_tensor("wgc", [WGC_PAD, DIM], f16, kind="ExternalInput").ap()
    ind_ap = nc.dram_tensor("ind", [128, IND_COLS], f16, kind="ExternalInput").ap()
    out_ap = nc.dram_tensor("out", [TOK, N_BANDS * DIM], i8, kind="ExternalOutput").ap()
    ibmd_ap = nc.dram_tensor("ibmd", [1, N_BANDS * TOK], f16, kind="Internal").ap()

    chunk_slice = {}
    for si, (c0, c1, *_r) in enumerate(SLICES):
        for c in range(c0, c1):
            chunk_slice[c] = (si, c - c0)

    with tile.TileContext(nc) as tc:
        with (
            tc.tile_pool(name="const", bufs=1) as const_pool,
            tc.tile_pool(name="xbuf", bufs=1) as xbuf_pool,
            tc.tile_pool(name="outb", bufs=4) as out_pool,
            tc.tile_pool(name="psum", bufs=4, space="PSUM") as psum_pool,
        ):
            NSL = len(SLICES)
            XTs, WGs, XSQs, XPs, INVs, IBMs = [], [], [], [], [], []
            for si, (c0, c1, p0, p1) in enumerate(SLICES):
                nch, nb = c1 - c0, p1 - p0
                XTs.append(const_pool.tile([128, nch * TOK], f16, name=f"xt{si}"))
                WGs.append(const_pool.tile([128, nch * DIM], f16, name=f"wg{si}"))
                XSQs.append(xbuf_pool.tile([128, nch * TOK], f16, name=f"xq{si}"))
                XPs.append(xbuf_pool.tile([128, nch * TOK], f16, name=f"xp{si}"))
                INVs.append(xbuf_pool.tile([32, TOK], f32, name=f"nv{si}"))
                IBMs.append(xbuf_pool.tile([32, TOK], f16, name=f"ib{si}"))
            IND = const_pool.tile([128, IND_COLS], f16, name="ind")
            XSCb = xbuf_pool.tile([128, N_CHUNKS * TOK], f16, name="xsb")

            # PE warmup burst while inputs land (HAM -> 2.4GHz)
            WRM = xbuf_pool.tile([128, 512], f16, name="wrm")
            nc.vector.memset(WRM[:], 0.0)
            PSW = psum_pool.tile([128, 1024], f32, space="PSUM", name="ps")
            for _ in range(8):
                nc.tensor.matmul(PSW[:, 0:512], WRM[:, 0:128], WRM[:],
                                 start=True, stop=True)

            # ---- input DMAs: xt on sync; ind + wgc scatter on gpsimd
            # (wgc entries are interleaved with the gpsimd squares below so
            # the Pool queue serves both in need-order)
            for si, (c0, c1, p0, p1) in enumerate(SLICES):
                nc.sync.dma_start(XTs[si][:], xt_ap[:, c0 * TOK : c1 * TOK])
            nc.gpsimd.dma_start(IND[:], ind_ap[:, :])

            def emit_wgc(si):
                for (p0_, nr, c0_, nch_, src0, sstride) in WG_SCATTER:
                    s_, crel = chunk_slice[c0_]
                    if s_ != si:
                        continue
                    seg = wgc_ap[src0 : src0 + sstride * nch_, :]
                    src = seg.rearrange("(j i) c -> i j c", j=nch_)[0:nr, :, :]
                    dst = WGs[s_][p0_ : p0_ + nr, crel * DIM : (crel + nch_) * DIM]
                    dst = dst.rearrange("p (j c) -> p j c", j=nch_)
                    nc.gpsimd.dma_start(dst, src)

            for _si in range(len(SLICES)):
                emit_wgc(_si)

            # ---- norm chain -----------------------------------------------
            def emit_norm(si):
                c0, c1, p0, p1 = SLICES[si]
                nch, nb = c1 - c0, p1 - p0
                SQP = psum_pool.tile([32, TOK], f32, space="PSUM", name="ps")
                for cr in range(nch):
                    c = c0 + cr
                    nc.tensor.matmul(
                        SQP[0:nb, :],
                        IND[:, IND_OFF[c] : IND_OFF[c] + nb],
                        XSQs[si][:, cr * TOK : (cr + 1) * TOK],
                        start=(cr == 0), stop=(cr == nch - 1),
                        tile_position=(0, 0),
                    )
                nc.vector.reciprocal_approx_fast(INVs[si][0:nb, :], SQP[0:nb, :])
                with nc.allow_low_precision("inv norm in f16 (rel 5e-4)"):
                    nc.scalar.activation(IBMs[si][0:nb, :], INVs[si][0:nb, :],
                                         AF.Sqrt)
                # flatten [nb, 256] -> one DRAM row at packed offset
                dstf = ibmd_ap[0:1, p0 * TOK : p1 * TOK].rearrange(
                    "p (b t) -> (p b) t", b=nb)
                nc.sync.dma_start(dstf, IBMs[si][0:nb, :])

            # inv-norm broadcast DMAs: IBMF row -> XSCb [feat_row, tok] image
            def emit_bcast(si):
                c0, c1, p0, p1 = SLICES[si]
                nch, nb = c1 - c0, p1 - p0
                if si == 0:
                    # chunks 0-3 = bands 60,61 each twice (dup chunks)
                    for bb in range(2):
                        src = ibmd_ap[0:1, bb * TOK : (bb + 1) * TOK]
                        src = src.unsqueeze(1).broadcast_to([128, 2, TOK])
                        dst = XSCb[:, (2 * bb) * TOK : (2 * bb + 2) * TOK]
                        dst = dst.rearrange("q (d t) -> q d t", d=2)
                        nc.sync.dma_start(dst, src)
                    return
                segs = CHUNKS[c0]
                nseg = len(segs)
                # packed pos = p0 + chunk_rel*nseg + seg_idx
                view = ibmd_ap[0:1, p0 * TOK : p1 * TOK].rearrange(
                    "p (j s t) -> p j s t", j=nch, s=nseg)
                for k, (slot, n, b0, _soff) in enumerate(segs):
                    nxt = segs[k + 1][0] if k + 1 < nseg else 128
                    nparts = nxt - slot  # cover pad rows too (finite values)
                    src = view[:, :, k : k + 1, :].squeeze(2).squeeze(0)
                    src = src.unsqueeze(0).broadcast_to([nparts, nch, TOK])
                    dst = XSCb[slot : slot + nparts,
                               c0 * TOK : c1 * TOK].rearrange(
                        "q (j t) -> q j t", j=nch)
                    nc.sync.dma_start(dst, src)

            def emit_prescale(si):
                c0, c1, p0, p1 = SLICES[si]
                with nc.allow_low_precision("prescaled x in f16"):
                    nc.vector.tensor_mul(
                        XPs[si][:], XTs[si][:],
                        XSCb[:, c0 * TOK : c1 * TOK])



            # ---- band matmuls + evac + out --------------------------------
            out_tiles = {}

            def out_tile_for(p, t):
                for r0, r1 in OUT_RANGES:
                    if r0 <= p < r1:
                        key = (r0, t)
                        if key not in out_tiles:
                            ot = out_pool.tile([128, (r1 - r0) * DIM], i8, name="ot")
                            out_tiles[key] = [ot, r0, r1, 0]
                        return out_tiles[key]
                raise AssertionError

            eng_load = {"dve": 6600.0, "act": 2750.0}

            def evac_quad(q0, nq, PSG, t):
                ent = out_tile_for(q0, t)
                ot, r0, r1, _ = ent
                dst = ot[:, (q0 - r0) * DIM : (q0 - r0 + nq) * DIM]
                src = PSG[:, 0 : nq * DIM]
                fd = nq * DIM
                cost = {"dve": fd * 1.042 + 160.0, "act": fd * 0.833 + 260.0}
                e = min(eng_load, key=lambda k: eng_load[k] + cost[k])
                eng_load[e] += cost[e]
                if e == "dve":
                    nc.vector.tensor_copy(dst, src)
                else:
                    nc.scalar.activation(dst, src, AF.Copy)
                ent[3] += nq
                if ent[3] == r1 - r0:
                    nc.sync.dma_start(
                        out_ap[t * 128 : (t + 1) * 128, r0 * DIM : r1 * DIM], ot[:]
                    )

            def emit_bands(si):
                c0, c1, p0, p1 = SLICES[si]
                for t in range(N_TILES):
                    for q0 in range(p0, p1, 2):
                        nq = min(2, p1 - q0)
                        PSG = psum_pool.tile([128, 1024], f32, space="PSUM",
                                             name="ps")
                        for k in range(nq):
                            b = PBANDS[q0 + k]
                            segs = PLACEMENT[b]
                            for sk, (c, slot, n, _src) in enumerate(segs):
                                sj, crel = chunk_slice[c]
                                nc.tensor.matmul(
                                    PSG[:, k * DIM : (k + 1) * DIM],
                                    XPs[sj][slot : slot + n,
                                            crel * TOK + t * 128
                                            : crel * TOK + t * 128 + 128],
                                    WGs[sj][slot : slot + n,
                                            crel * DIM : (crel + 1) * DIM],
                                    start=(sk == 0), stop=(sk == len(segs) - 1),
                                    tile_position=(slot, 0),
                                )
                        evac_quad(q0, nq, PSG, t)

            # ---- pipeline: stage-parallel emission.  All squares first so
            # the PE's ssq chain never stalls mid-FIFO; per-slice norm
            # chains pipeline across engines; prescales run 2 slices ahead
            # of their band matmuls.
            for si in range(NSL):
                nc.vector.tensor_mul(XSQs[si][:], XTs[si][:], XTs[si][:])
            for si in range(NSL):
                emit_norm(si)
                emit_bcast(si)
            emit_prescale(0)
            emit_prescale(1)
            for si in range(NSL):
                if si + 2 < NSL:
                    emit_prescale(si + 2)
                emit_bands(si)

    nc.compile()
    return nc


def _get_program():
    if "nc" not in _CACHE:
        _CACHE["nc"] = _build_program()
    return _CACHE["nc"]


def _prep_weights(gamma, W):
    """fold gamma + sqrt(d) + per-band int8 scale into W; compact wgc rows."""
    wf = np.asarray(W, dtype=np.float32) * np.asarray(gamma, dtype=np.float32)[:, None]
    deq = np.empty((N_BANDS,), dtype=np.float32)
    wq = np.empty_like(wf)
    for b, d in enumerate(DIM_INPUTS):
        o = OFFSETS[b]
        wb = wf[o : o + d]
        m = max(float(np.sqrt((wb * wb).sum(axis=0)).max()), 1e-30)
        s = ZQ * m / (127.0 * np.sqrt(d))
        wq[o : o + d] = wb / s
        deq[b] = s * np.sqrt(d)
    wgc = np.zeros((WGC_PAD, DIM), dtype=np.float16)
    wgc[:F_TOTAL] = wq[WGC_ROWS].astype(np.float16)
    return wgc, deq


def _host_const():
    """ssq indicator [128, IND_COLS]."""
    ppos = {b: p for p, b in enumerate(PBANDS)}
    slice_p0 = {}
    for (c0, c1, p0, p1) in SLICES:
        for c in range(c0, c1):
            slice_p0[c] = p0
    ind = np.zeros((128, IND_COLS), dtype=np.float16)
    for c, segs in enumerate(CHUNKS):
        for (slot, n, bb, soff) in segs:
            brel = ppos[bb] - slice_p0[c]
            ind[slot : slot + n, IND_OFF[c] + brel] = 1.0
    return ind


def _host_inputs(x, gamma, W):
    xf = np.ascontiguousarray(np.asarray(x, dtype=np.float32).reshape(BT, F_TOTAL))
    wgc, deq = _prep_weights(gamma, W)
    ind = _host_const()

    valid = ROW_MAP >= 0
    src_rows = ROW_MAP[valid]
    in_maps = []
    for i in range(N_CORES):
        shard = xf[i * TOK : (i + 1) * TOK]
        xtp = np.zeros((F_PACK, TOK), dtype=np.float32)
        xtp[valid] = shard.T[src_rows]
        xtp = np.ascontiguousarray(
            xtp.astype(np.float16).reshape(N_CHUNKS, 128, TOK).transpose(1, 0, 2)
        ).reshape(128, N_CHUNKS * TOK)
        in_maps.append({"xt": xtp, "wgc": wgc, "ind": ind})
    return in_maps, deq


def _unpack_out(packed_list, deq, b):
    bf = np.asarray(b, dtype=np.float32)
    pb = np.asarray(PBANDS)
    scale_packed = deq[pb]
    out = np.empty((BT, N_BANDS, DIM), dtype=np.float32)
    for i, pk in enumerate(packed_list):
        pk = pk.reshape(TOK, N_BANDS, DIM)
        out[i * TOK : (i + 1) * TOK, pb] = (
            pk.astype(np.float32) * scale_packed[None, :, None]
        )
    out = out.reshape(B, T, N_BANDS, DIM)
    return out + bf[None, None, :, :]


def _run(x, gamma, W, b, trace=False, trace_kwargs=None):
    nc = _get_program()
    in_maps, deq = _host_inputs(x, gamma, W)
    kw = {}
    if trace:
        kw = {"trace": True, "trace_kwargs": trace_kwargs or {}}
    res = run_bass_kernel_spmd(nc, in_maps, core_ids=list(range(N_CORES)), **kw)
    out = _unpack_out([res.results[i]["out"] for i in range(N_CORES)], deq, b)
    return out, res


def kernel(x, gamma, W, b):
    out, _ = _run(x, gamma, W, b)
    return out


# revision 8
# speedup vs baseline: 1.1461x; 1.1461x over previous
"""BandSplitEncoder Trainium2 kernel, v3.

x[B,T,2048] -> 62 band RMSNorm+Linear[d->512] -> [B,T,62,512].
Data-parallel over the 2048 B*T tokens across 8 NeuronCores (256 each).

Per core:
  - x ships as a zero-padded packed fp16 image [128, 25*256] (25 chunks
    of 128 feature rows, bands slot-aligned, heavy bands first).
  - norm chain (all slices up front): DVE squares (f16 2x) -> per-chunk
    indicator matmuls reduce the partition dim -> ssq[band, tok] f32
    PSUM -> DVE reciprocal_approx_fast -> ACT sqrt -> inv[band, tok]
    f16 -> one SBUF->SBUF DMA flattens it to a single partition row ->
    13 partition-broadcast DMAs expand it to the [feat_row, tok] image
    -> DVE multiply (f16 2x) prescales x.
  - band matmuls (W carries gamma, sqrt(d) and the int8 scale, so PSUM
    holds final int8 code values): 4 packed bands per [128,2048] f32
    PSUM quad (4 banks, one pool bufs=2 = all 8 banks); small-K bands
    run concurrently via tile_position row groups.
  - evac = pure f32->int8 saturating copy per quad, split ACT/DVE by a
    measured-cost greedy; int8 out tiles DMA per packed-band range.
  - W ships compact [2080,512] f16 (chunk-consumption row order),
    scattered to the slot layout by 13 strided DMAs on the GpSimd
    queue; pad rows are never read.
Host folds gamma + sqrt(d) + per-band int8 scale into W, packs x, and
dequantizes + adds bias on the way out.
"""

import numpy as np

import concourse.bacc as bacc
import concourse.tile as tile
from concourse import mybir
from concourse.bass_utils import run_bass_kernel_spmd

# ---------------------------------------------------------------- problem dims
DIM_INPUTS = (4,) * 24 + (8,) * 12 + (24,) * 8 + (48,) * 8 + (96,) * 8 + (256,) * 2
N_BANDS = len(DIM_INPUTS)  # 62
F_TOTAL = sum(DIM_INPUTS)  # 2048
DIM = 512
B, T = 4, 512
BT = B * T
N_CORES = 8
TOK = BT // N_CORES  # 256
N_TILES = TOK // 128  # 2
ZQ = 6.0  # int8 quant margin in units of the per-band max W column norm
WGC_PAD = 2080  # wgc DRAM rows incl. slack so strided scatter APs stay in-bounds

OFFSETS = []
_off = 0
for _d in DIM_INPUTS:
    OFFSETS.append(_off)
    _off += _d

# ------------------------------------------------- packed chunk layout
CHUNKS = []
for b in (60, 61):  # d256: two full chunks each
    CHUNKS.append([(0, 128, b, 0)])
    CHUNKS.append([(0, 128, b, 128)])
for i in range(44, 52, 2):  # d48: two per chunk at slots 0/64
    CHUNKS.append([(0, 48, i, 0), (64, 48, i + 1, 0)])
for k in range(8):  # d96 at slot 0 + d24 at slot 96
    CHUNKS.append([(0, 96, 52 + k, 0), (96, 24, 36 + k, 0)])
for i in range(24, 36, 4):  # d8: four per chunk
    CHUNKS.append([(32 * j, 8, i + j, 0) for j in range(4)])
for i in range(0, 24, 4):  # d4: four per chunk
    CHUNKS.append([(32 * j, 4, i + j, 0) for j in range(4)])
N_CHUNKS = len(CHUNKS)  # 25
F_PACK = N_CHUNKS * 128  # 3200

PLACEMENT = [[] for _ in range(N_BANDS)]
PBANDS = []
for _c, _segs in enumerate(CHUNKS):
    for _slot, _n, _b, _soff in _segs:
        PLACEMENT[_b].append((_c, _slot, _n, OFFSETS[_b] + _soff))
        if _b not in PBANDS:
            PBANDS.append(_b)

ROW_MAP = np.full((F_PACK,), -1, dtype=np.int64)
for _b in range(N_BANDS):
    for _c, _slot, _n, _src in PLACEMENT[_b]:
        ROW_MAP[_c * 128 + _slot : _c * 128 + _slot + _n] = np.arange(_src, _src + _n)

WGC_ROWS = np.concatenate(
    [np.arange(OFFSETS[b] + soff, OFFSETS[b] + soff + n)
     for segs in CHUNKS for (slot, n, b, soff) in segs]
)
assert WGC_ROWS.shape[0] == F_TOTAL

# pipeline slices: (c0, c1, p0, p1) chunk/packed-band ranges
SLICES = [(0, 4, 0, 2), (4, 8, 2, 10), (8, 16, 10, 26),
          (16, 19, 26, 38), (19, 25, 38, 62)]

# out tiles: packed-band ranges (quad-aligned)
OUT_RANGES = [(0, 10), (10, 26), (26, 38), (38, 50), (50, 62)]

# WG scatter specs: (dst_part0, nrows, c0, nch, src_row0, src_chunk_stride)
WG_SCATTER = (
    [(0, 128, 0, 4, 0, 128),
     (0, 48, 4, 4, 512, 96), (64, 48, 4, 4, 560, 96),
     (0, 96, 8, 8, 896, 120), (96, 24, 8, 8, 992, 120)]
    + [(32 * s, 8, 16, 3, 1856 + 8 * s, 32) for s in range(4)]
    + [(32 * s, 4, 19, 6, 1952 + 4 * s, 16) for s in range(4)]
)

# inv-norm broadcast specs per slice:
# (part0, nparts, band_step_rows, first_band_rel, nch_grp, chunk_band_stride)
# dst = XSCb[part0:part0+nparts, (c0+j)*256 : ...] for j in 0..nch_grp
# src row (packed pos) = p0 + first_band_rel + j*chunk_band_stride
BCASTS = {
    0: [(0, 128, 0, [(0, 2, 0)])],  # handled specially (dup chunks)
}

# indicator column offsets per chunk
IND_OFF = []
_io = 0
for _c in range(N_CHUNKS):
    for (c0, c1, p0, p1) in SLICES:
        if c0 <= _c < c1:
            IND_OFF.append(_io)
            _io += p1 - p0
IND_COLS = _io  # 348

_CACHE = {}


def _build_program():
    nc = bacc.Bacc("TRN2", target_bir_lowering=False, debug=False, num_devices=N_CORES)
    f32 = mybir.dt.float32
    f16 = mybir.dt.float16
    i8 = mybir.dt.int8
    AF = mybir.ActivationFunctionType

    xt_ap = nc.dram_tensor("xt", [128, N_CHUNKS * TOK], f16, kind="ExternalInput").ap()
    wgc_ap = nc.dram_tensor("wgc", [WGC_PAD, DIM], f16, kind="ExternalInput").ap()
    ind_ap = nc.dram_tensor("ind", [128, IND_COLS], f16, kind="ExternalInput").ap()
    out_ap = nc.dram_tensor("out", [TOK, N_BANDS * DIM], i8, kind="ExternalOutput").ap()
    ibmd_ap = nc.dram_tensor("ibmd", [1, N_BANDS * TOK], f16, kind="Internal").ap()

    chunk_slice = {}
    for si, (c0, c1, *_r) in enumerate(SLICES):
        for c in range(c0, c1):
            chunk_slice[c] = (si, c - c0)

    with tile.TileContext(nc) as tc:
        with (
            tc.tile_pool(name="const", bufs=1) as const_pool,
            tc.tile_pool(name="xbuf", bufs=1) as xbuf_pool,
            tc.tile_pool(name="outb", bufs=6) as out_pool,
            tc.tile_pool(name="psum", bufs=4, space="PSUM") as psum_pool,
        ):
            NSL = len(SLICES)
            XTs, WGs, XSQs, XPs, INVs, IBMs = [], [], [], [], [], []
            for si, (c0, c1, p0, p1) in enumerate(SLICES):
                nch, nb = c1 - c0, p1 - p0
                XTs.append(const_pool.tile([128, nch * TOK], f16, name=f"xt{si}"))
                WGs.append(const_pool.tile([128, nch * DIM], f16, name=f"wg{si}"))
                XSQs.append(xbuf_pool.tile([128, nch * TOK], f16, name=f"xq{si}"))
                XPs.append(xbuf_pool.tile([128, nch * TOK], f16, name=f"xp{si}"))
                INVs.append(xbuf_pool.tile([32, TOK], f32, name=f"nv{si}"))
                IBMs.append(xbuf_pool.tile([32, TOK], f16, name=f"ib{si}"))
            IND = const_pool.tile([128, IND_COLS], f16, name="ind")
            XSCb = xbuf_pool.tile([128, N_CHUNKS * TOK], f16, name="xsb")

            # PE warmup burst while inputs land (HAM -> 2.4GHz)
            WRM = xbuf_pool.tile([128, 512], f16, name="wrm")
            nc.vector.memset(WRM[:], 0.0)
            PSW = psum_pool.tile([128, 1024], f32, space="PSUM", name="ps")
            for _ in range(10):
                nc.tensor.matmul(PSW[:, 0:512], WRM[:, 0:128], WRM[:],
                                 start=True, stop=True)

            # ---- input DMAs: xt on sync; ind + wgc scatter on gpsimd
            # (wgc entries are interleaved with the gpsimd squares below so
            # the Pool queue serves both in need-order)
            for si, (c0, c1, p0, p1) in enumerate(SLICES):
                nc.sync.dma_start(XTs[si][:], xt_ap[:, c0 * TOK : c1 * TOK])
            nc.gpsimd.dma_start(IND[:], ind_ap[:, :])

            def emit_wgc(si):
                for (p0_, nr, c0_, nch_, src0, sstride) in WG_SCATTER:
                    s_, crel = chunk_slice[c0_]
                    if s_ != si:
                        continue
                    seg = wgc_ap[src0 : src0 + sstride * nch_, :]
                    src = seg.rearrange("(j i) c -> i j c", j=nch_)[0:nr, :, :]
                    dst = WGs[s_][p0_ : p0_ + nr, crel * DIM : (crel + nch_) * DIM]
                    dst = dst.rearrange("p (j c) -> p j c", j=nch_)
                    nc.sync.dma_start(dst, src)

            for _si in range(len(SLICES)):
                emit_wgc(_si)

            # ---- norm chain -----------------------------------------------
            def emit_norm(si):
                c0, c1, p0, p1 = SLICES[si]
                nch, nb = c1 - c0, p1 - p0
                nc.vector.tensor_mul(XSQs[si][:], XTs[si][:], XTs[si][:])
                SQP = psum_pool.tile([32, TOK], f32, space="PSUM", name="ps")
                for cr in range(nch):
                    c = c0 + cr
                    nc.tensor.matmul(
                        SQP[0:nb, :],
                        IND[:, IND_OFF[c] : IND_OFF[c] + nb],
                        XSQs[si][:, cr * TOK : (cr + 1) * TOK],
                        start=(cr == 0), stop=(cr == nch - 1),
                        tile_position=(0, 0),
                    )
                nc.vector.reciprocal_approx_fast(INVs[si][0:nb, :], SQP[0:nb, :])
                with nc.allow_low_precision("inv norm in f16 (rel 5e-4)"):
                    nc.scalar.activation(IBMs[si][0:nb, :], INVs[si][0:nb, :],
                                         AF.Sqrt)
                # flatten [nb, 256] -> one DRAM row at packed offset
                dstf = ibmd_ap[0:1, p0 * TOK : p1 * TOK].rearrange(
                    "p (b t) -> (p b) t", b=nb)
                nc.gpsimd.dma_start(dstf, IBMs[si][0:nb, :])

            # inv-norm broadcast DMAs: IBMF row -> XSCb [feat_row, tok] image
            def emit_bcast(si):
                c0, c1, p0, p1 = SLICES[si]
                nch, nb = c1 - c0, p1 - p0
                if si == 0:
                    # chunks 0-3 = bands 60,61 each twice (dup chunks)
                    for bb in range(2):
                        src = ibmd_ap[0:1, bb * TOK : (bb + 1) * TOK]
                        src = src.unsqueeze(1).broadcast_to([128, 2, TOK])
                        dst = XSCb[:, (2 * bb) * TOK : (2 * bb + 2) * TOK]
                        dst = dst.rearrange("q (d t) -> q d t", d=2)
                        nc.gpsimd.dma_start(dst, src)
                    return
                segs = CHUNKS[c0]
                nseg = len(segs)
                # packed pos = p0 + chunk_rel*nseg + seg_idx
                view = ibmd_ap[0:1, p0 * TOK : p1 * TOK].rearrange(
                    "p (j s t) -> p j s t", j=nch, s=nseg)
                for k, (slot, n, b0, _soff) in enumerate(segs):
                    nxt = segs[k + 1][0] if k + 1 < nseg else 128
                    nparts = nxt - slot  # cover pad rows too (finite values)
                    src = view[:, :, k : k + 1, :].squeeze(2).squeeze(0)
                    src = src.unsqueeze(0).broadcast_to([nparts, nch, TOK])
                    dst = XSCb[slot : slot + nparts,
                               c0 * TOK : c1 * TOK].rearrange(
                        "q (j t) -> q j t", j=nch)
                    nc.gpsimd.dma_start(dst, src)

            def emit_prescale(si):
                c0, c1, p0, p1 = SLICES[si]
                with nc.allow_low_precision("prescaled x in f16"):
                    nc.vector.tensor_mul(
                        XPs[si][:], XTs[si][:],
                        XSCb[:, c0 * TOK : c1 * TOK])



            # ---- band matmuls + evac + out --------------------------------
            out_tiles = {}

            def out_tile_for(p, t):
                for r0, r1 in OUT_RANGES:
                    if r0 <= p < r1:
                        key = (r0, t)
                        if key not in out_tiles:
                            ot = out_pool.tile([128, (r1 - r0) * DIM], i8, name="ot")
                            out_tiles[key] = [ot, r0, r1, 0]
                        return out_tiles[key]
                raise AssertionError

            eng_load = {"dve": 6600.0, "act": 2750.0}

            def evac_quad(q0, nq, PSG, t):
                ent = out_tile_for(q0, t)
                ot, r0, r1, _ = ent
                dst = ot[:, (q0 - r0) * DIM : (q0 - r0 + nq) * DIM]
                src = PSG[:, 0 : nq * DIM]
                fd = nq * DIM
                cost = {"dve": fd * 1.042 + 160.0, "act": fd * 0.833 + 260.0}
                e = min(eng_load, key=lambda k: eng_load[k] + cost[k])
                eng_load[e] += cost[e]
                if e == "dve":
                    nc.vector.tensor_copy(dst, src)
                else:
                    nc.scalar.activation(dst, src, AF.Copy)
                ent[3] += nq
                if ent[3] == r1 - r0:
                    nc.sync.dma_start(
                        out_ap[t * 128 : (t + 1) * 128, r0 * DIM : r1 * DIM], ot[:]
                    )

            def emit_bands(si):
                c0, c1, p0, p1 = SLICES[si]
                for t in range(N_TILES):
                    for q0 in range(p0, p1, 2):
                        nq = min(2, p1 - q0)
                        PSG = psum_pool.tile([128, 1024], f32, space="PSUM",
                                             name="ps")
                        for k in range(nq):
                            b = PBANDS[q0 + k]
                            segs = PLACEMENT[b]
                            for sk, (c, slot, n, _src) in enumerate(segs):
                                sj, crel = chunk_slice[c]
                                nc.tensor.matmul(
                                    PSG[:, k * DIM : (k + 1) * DIM],
                                    XPs[sj][slot : slot + n,
                                            crel * TOK + t * 128
                                            : crel * TOK + t * 128 + 128],
                                    WGs[sj][slot : slot + n,
                                            crel * DIM : (crel + 1) * DIM],
                                    start=(sk == 0), stop=(sk == len(segs) - 1),
                                    tile_position=(slot, 0),
                                )
                        evac_quad(q0, nq, PSG, t)

            # ---- pipeline: 1-slice lookahead keeps every engine fed
            def emit_slice_norm(si):
                emit_norm(si)
                emit_bcast(si)
                emit_prescale(si)

            emit_slice_norm(0)
            emit_slice_norm(1)
            for si in range(NSL):
                if si + 2 < NSL:
                    emit_slice_norm(si + 2)
                emit_bands(si)

    nc.compile()
    return nc


def _get_program():
    if "nc" not in _CACHE:
        _CACHE["nc"] = _build_program()
    return _CACHE["nc"]


def _prep_weights(gamma, W):
    """fold gamma + sqrt(d) + per-band int8 scale into W; compact wgc rows."""
    wf = np.asarray(W, dtype=np.float32) * np.asarray(gamma, dtype=np.float32)[:, None]
    deq = np.empty((N_BANDS,), dtype=np.float32)
    wq = np.empty_like(wf)
    for b, d in enumerate(DIM_INPUTS):
        o = OFFSETS[b]
        wb = wf[o : o + d]
        m = max(float(np.sqrt((wb * wb).sum(axis=0)).max()), 1e-30)
        s = ZQ * m / (127.0 * np.sqrt(d))
        wq[o : o + d] = wb / s
        deq[b] = s * np.sqrt(d)
    wgc = np.zeros((WGC_PAD, DIM), dtype=np.float16)
    wgc[:F_TOTAL] = wq[WGC_ROWS].astype(np.float16)
    return wgc, deq


def _host_const():
    """ssq indicator [128, IND_COLS]."""
    ppos = {b: p for p, b in enumerate(PBANDS)}
    slice_p0 = {}
    for (c0, c1, p0, p1) in SLICES:
        for c in range(c0, c1):
            slice_p0[c] = p0
    ind = np.zeros((128, IND_COLS), dtype=np.float16)
    for c, segs in enumerate(CHUNKS):
        for (slot, n, bb, soff) in segs:
            brel = ppos[bb] - slice_p0[c]
            ind[slot : slot + n, IND_OFF[c] + brel] = 1.0
    return ind


def _host_inputs(x, gamma, W):
    xf = np.ascontiguousarray(np.asarray(x, dtype=np.float32).reshape(BT, F_TOTAL))
    wgc, deq = _prep_weights(gamma, W)
    ind = _host_const()

    valid = ROW_MAP >= 0
    src_rows = ROW_MAP[valid]
    in_maps = []
    for i in range(N_CORES):
        shard = xf[i * TOK : (i + 1) * TOK]
        xtp = np.zeros((F_PACK, TOK), dtype=np.float32)
        xtp[valid] = shard.T[src_rows]
        xtp = np.ascontiguousarray(
            xtp.astype(np.float16).reshape(N_CHUNKS, 128, TOK).transpose(1, 0, 2)
        ).reshape(128, N_CHUNKS * TOK)
        in_maps.append({"xt": xtp, "wgc": wgc, "ind": ind})
    return in_maps, deq


def _unpack_out(packed_list, deq, b):
    bf = np.asarray(b, dtype=np.float32)
    pb = np.asarray(PBANDS)
    scale_packed = deq[pb]
    out = np.empty((BT, N_BANDS, DIM), dtype=np.float32)
    for i, pk in enumerate(packed_list):
        pk = pk.reshape(TOK, N_BANDS, DIM)
        out[i * TOK : (i + 1) * TOK, pb] = (
            pk.astype(np.float32) * scale_packed[None, :, None]
        )
    out = out.reshape(B, T, N_BANDS, DIM)
    return out + bf[None, None, :, :]


def _run(x, gamma, W, b, trace=False, trace_kwargs=None):
    nc = _get_program()
    in_maps, deq = _host_inputs(x, gamma, W)
    kw = {}
    if trace:
        kw = {"trace": True, "trace_kwargs": trace_kwargs or {}}
    res = run_bass_kernel_spmd(nc, in_maps, core_ids=list(range(N_CORES)), **kw)
    out = _unpack_out([res.results[i]["out"] for i in range(N_CORES)], deq, b)
    return out, res


def kernel(x, gamma, W, b):
    out, _ = _run(x, gamma, W, b)
    return out


# revision 9
# speedup vs baseline: 1.1533x; 1.0063x over previous
"""BandSplitEncoder Trainium2 kernel, v3.

x[B,T,2048] -> 62 band RMSNorm+Linear[d->512] -> [B,T,62,512].
Data-parallel over the 2048 B*T tokens across 8 NeuronCores (256 each).

Per core:
  - x ships as a zero-padded packed fp16 image [128, 25*256] (25 chunks
    of 128 feature rows, bands slot-aligned, heavy bands first).
  - norm chain (all slices up front): DVE squares (f16 2x) -> per-chunk
    indicator matmuls reduce the partition dim -> ssq[band, tok] f32
    PSUM -> DVE reciprocal_approx_fast -> ACT sqrt -> inv[band, tok]
    f16 -> one SBUF->SBUF DMA flattens it to a single partition row ->
    13 partition-broadcast DMAs expand it to the [feat_row, tok] image
    -> DVE multiply (f16 2x) prescales x.
  - band matmuls (W carries gamma, sqrt(d) and the int8 scale, so PSUM
    holds final int8 code values): 4 packed bands per [128,2048] f32
    PSUM quad (4 banks, one pool bufs=2 = all 8 banks); small-K bands
    run concurrently via tile_position row groups.
  - evac = pure f32->int8 saturating copy per quad, split ACT/DVE by a
    measured-cost greedy; int8 out tiles DMA per packed-band range.
  - W ships compact [2080,512] f16 (chunk-consumption row order),
    scattered to the slot layout by 13 strided DMAs on the GpSimd
    queue; pad rows are never read.
Host folds gamma + sqrt(d) + per-band int8 scale into W, packs x, and
dequantizes + adds bias on the way out.
"""

import numpy as np

import concourse.bacc as bacc
import concourse.tile as tile
from concourse import mybir
from concourse.bass_utils import run_bass_kernel_spmd

# ---------------------------------------------------------------- problem dims
DIM_INPUTS = (4,) * 24 + (8,) * 12 + (24,) * 8 + (48,) * 8 + (96,) * 8 + (256,) * 2
N_BANDS = len(DIM_INPUTS)  # 62
F_TOTAL = sum(DIM_INPUTS)  # 2048
DIM = 512
B, T = 4, 512
BT = B * T
N_CORES = 8
TOK = BT // N_CORES  # 256
N_TILES = TOK // 128  # 2
ZQ = 6.0  # int8 quant margin in units of the per-band max W column norm
WGC_PAD = 2080  # wgc DRAM rows incl. slack so strided scatter APs stay in-bounds

OFFSETS = []
_off = 0
for _d in DIM_INPUTS:
    OFFSETS.append(_off)
    _off += _d

# ------------------------------------------------- packed chunk layout
CHUNKS = []
for b in (60, 61):  # d256: two full chunks each
    CHUNKS.append([(0, 128, b, 0)])
    CHUNKS.append([(0, 128, b, 128)])
for i in range(44, 52, 2):  # d48: two per chunk at slots 0/64
    CHUNKS.append([(0, 48, i, 0), (64, 48, i + 1, 0)])
for k in range(8):  # d96 at slot 0 + d24 at slot 96
    CHUNKS.append([(0, 96, 52 + k, 0), (96, 24, 36 + k, 0)])
for i in range(24, 36, 4):  # d8: four per chunk
    CHUNKS.append([(32 * j, 8, i + j, 0) for j in range(4)])
for i in range(0, 24, 4):  # d4: four per chunk
    CHUNKS.append([(32 * j, 4, i + j, 0) for j in range(4)])
N_CHUNKS = len(CHUNKS)  # 25
F_PACK = N_CHUNKS * 128  # 3200

PLACEMENT = [[] for _ in range(N_BANDS)]
PBANDS = []
for _c, _segs in enumerate(CHUNKS):
    for _slot, _n, _b, _soff in _segs:
        PLACEMENT[_b].append((_c, _slot, _n, OFFSETS[_b] + _soff))
        if _b not in PBANDS:
            PBANDS.append(_b)

ROW_MAP = np.full((F_PACK,), -1, dtype=np.int64)
for _b in range(N_BANDS):
    for _c, _slot, _n, _src in PLACEMENT[_b]:
        ROW_MAP[_c * 128 + _slot : _c * 128 + _slot + _n] = np.arange(_src, _src + _n)

WGC_ROWS = np.concatenate(
    [np.arange(OFFSETS[b] + soff, OFFSETS[b] + soff + n)
     for segs in CHUNKS for (slot, n, b, soff) in segs]
)
assert WGC_ROWS.shape[0] == F_TOTAL

# pipeline slices: (c0, c1, p0, p1) chunk/packed-band ranges
SLICES = [(0, 4, 0, 2), (4, 8, 2, 10), (8, 16, 10, 26),
          (16, 19, 26, 38), (19, 25, 38, 62)]

# out tiles: packed-band ranges (quad-aligned)
OUT_RANGES = [(0, 10), (10, 26), (26, 38), (38, 50), (50, 62)]

# WG scatter specs: (dst_part0, nrows, c0, nch, src_row0, src_chunk_stride)
WG_SCATTER = (
    [(0, 128, 0, 4, 0, 128),
     (0, 48, 4, 4, 512, 96), (64, 48, 4, 4, 560, 96),
     (0, 96, 8, 8, 896, 120), (96, 24, 8, 8, 992, 120)]
    + [(32 * s, 8, 16, 3, 1856 + 8 * s, 32) for s in range(4)]
    + [(32 * s, 4, 19, 6, 1952 + 4 * s, 16) for s in range(4)]
)

# inv-norm broadcast specs per slice:
# (part0, nparts, band_step_rows, first_band_rel, nch_grp, chunk_band_stride)
# dst = XSCb[part0:part0+nparts, (c0+j)*256 : ...] for j in 0..nch_grp
# src row (packed pos) = p0 + first_band_rel + j*chunk_band_stride
BCASTS = {
    0: [(0, 128, 0, [(0, 2, 0)])],  # handled specially (dup chunks)
}

# indicator column offsets per chunk
IND_OFF = []
_io = 0
for _c in range(N_CHUNKS):
    for (c0, c1, p0, p1) in SLICES:
        if c0 <= _c < c1:
            IND_OFF.append(_io)
            _io += p1 - p0
IND_COLS = _io  # 348

_CACHE = {}


def _build_program():
    nc = bacc.Bacc("TRN2", target_bir_lowering=False, debug=False, num_devices=N_CORES)
    f32 = mybir.dt.float32
    f16 = mybir.dt.float16
    i8 = mybir.dt.int8
    AF = mybir.ActivationFunctionType

    xt_ap = nc.dram_tensor("xt", [128, N_CHUNKS * TOK], f16, kind="ExternalInput").ap()
    wgc_ap = nc.dram_tensor("wgc", [WGC_PAD, DIM], f16, kind="ExternalInput").ap()
    ind_ap = nc.dram_tensor("ind", [128, IND_COLS], f16, kind="ExternalInput").ap()
    out_ap = nc.dram_tensor("out", [TOK, N_BANDS * DIM], i8, kind="ExternalOutput").ap()
    indt_ap = nc.dram_tensor("indt", [32, N_CHUNKS * 128], f16,
                             kind="ExternalInput").ap()

    chunk_slice = {}
    for si, (c0, c1, *_r) in enumerate(SLICES):
        for c in range(c0, c1):
            chunk_slice[c] = (si, c - c0)

    with tile.TileContext(nc) as tc:
        with (
            tc.tile_pool(name="const", bufs=1) as const_pool,
            tc.tile_pool(name="xbuf", bufs=1) as xbuf_pool,
            tc.tile_pool(name="outb", bufs=6) as out_pool,
            tc.tile_pool(name="psum", bufs=4, space="PSUM") as psum_pool,
        ):
            NSL = len(SLICES)
            XTs, WGs, XSQs, XPs, INVs, IBMs = [], [], [], [], [], []
            for si, (c0, c1, p0, p1) in enumerate(SLICES):
                nch, nb = c1 - c0, p1 - p0
                XTs.append(const_pool.tile([128, nch * TOK], f16, name=f"xt{si}"))
                WGs.append(const_pool.tile([128, nch * DIM], f16, name=f"wg{si}"))
                XSQs.append(xbuf_pool.tile([128, nch * TOK], f16, name=f"xq{si}"))
                XPs.append(xbuf_pool.tile([128, nch * TOK], f16, name=f"xp{si}"))
                INVs.append(xbuf_pool.tile([32, TOK], f32, name=f"nv{si}"))
                IBMs.append(xbuf_pool.tile([32, TOK], f16, name=f"ib{si}"))
            IND = const_pool.tile([128, IND_COLS], f16, name="ind")
            INDT = const_pool.tile([32, N_CHUNKS * 128], f16, name="indt")

            # PE warmup burst while inputs land (HAM -> 2.4GHz)
            WRM = xbuf_pool.tile([128, 512], f16, name="wrm")
            nc.vector.memset(WRM[:], 0.0)
            PSW = psum_pool.tile([128, 1024], f32, space="PSUM", name="ps")
            for _ in range(10):
                nc.tensor.matmul(PSW[:, 0:512], WRM[:, 0:128], WRM[:],
                                 start=True, stop=True)

            # ---- input DMAs: xt on sync; ind + wgc scatter on gpsimd
            # (wgc entries are interleaved with the gpsimd squares below so
            # the Pool queue serves both in need-order)
            for si, (c0, c1, p0, p1) in enumerate(SLICES):
                nc.sync.dma_start(XTs[si][:], xt_ap[:, c0 * TOK : c1 * TOK])
            nc.gpsimd.dma_start(IND[:], ind_ap[:, :])
            nc.gpsimd.dma_start(INDT[:], indt_ap[:, :])

            def emit_wgc(si):
                for (p0_, nr, c0_, nch_, src0, sstride) in WG_SCATTER:
                    s_, crel = chunk_slice[c0_]
                    if s_ != si:
                        continue
                    seg = wgc_ap[src0 : src0 + sstride * nch_, :]
                    src = seg.rearrange("(j i) c -> i j c", j=nch_)[0:nr, :, :]
                    dst = WGs[s_][p0_ : p0_ + nr, crel * DIM : (crel + nch_) * DIM]
                    dst = dst.rearrange("p (j c) -> p j c", j=nch_)
                    nc.sync.dma_start(dst, src)

            for _si in range(len(SLICES)):
                emit_wgc(_si)

            # ---- norm chain -----------------------------------------------
            def emit_norm(si):
                c0, c1, p0, p1 = SLICES[si]
                nch, nb = c1 - c0, p1 - p0
                nc.vector.tensor_mul(XSQs[si][:], XTs[si][:], XTs[si][:])
                SQP = psum_pool.tile([32, TOK], f32, space="PSUM", name="ps")
                for cr in range(nch):
                    c = c0 + cr
                    nc.tensor.matmul(
                        SQP[0:nb, :],
                        IND[:, IND_OFF[c] : IND_OFF[c] + nb],
                        XSQs[si][:, cr * TOK : (cr + 1) * TOK],
                        start=(cr == 0), stop=(cr == nch - 1),
                        tile_position=(0, 0),
                    )
                nc.vector.reciprocal_approx_fast(INVs[si][0:nb, :], SQP[0:nb, :])
                with nc.allow_low_precision("inv norm in f16 (rel 5e-4)"):
                    nc.scalar.activation(IBMs[si][0:nb, :], INVs[si][0:nb, :],
                                         AF.Sqrt)

            # expand inv (band-major) back to [feat_row, tok] via PE
            # indicator matmuls, then prescale x straight from PSUM (DVE)
            def emit_prescale(si):
                c0, c1, p0, p1 = SLICES[si]
                nch, nb = c1 - c0, p1 - p0
                for u0 in range(0, nch, 4):
                    un = min(4, nch - u0)
                    XSC = psum_pool.tile([128, 1024], f32, space="PSUM",
                                         name="ps")
                    for j in range(un):
                        c = c0 + u0 + j
                        nc.tensor.matmul(
                            XSC[:, j * TOK : (j + 1) * TOK],
                            INDT[0:nb, c * 128 : (c + 1) * 128],
                            IBMs[si][0:nb, :],
                            start=True, stop=True,
                            tile_position=(0, 0),
                        )
                    with nc.allow_low_precision("prescaled x in f16"):
                        nc.vector.tensor_mul(
                            XPs[si][:, u0 * TOK : (u0 + un) * TOK],
                            XTs[si][:, u0 * TOK : (u0 + un) * TOK],
                            XSC[:, 0 : un * TOK],
                        )

            # ---- band matmuls + evac + out --------------------------------
            out_tiles = {}

            def out_tile_for(p, t):
                for r0, r1 in OUT_RANGES:
                    if r0 <= p < r1:
                        key = (r0, t)
                        if key not in out_tiles:
                            ot = out_pool.tile([128, (r1 - r0) * DIM], i8, name="ot")
                            out_tiles[key] = [ot, r0, r1, 0]
                        return out_tiles[key]
                raise AssertionError

            eng_load = {"dve": 17300.0, "act": 2750.0}

            def evac_quad(q0, nq, PSG, t):
                ent = out_tile_for(q0, t)
                ot, r0, r1, _ = ent
                dst = ot[:, (q0 - r0) * DIM : (q0 - r0 + nq) * DIM]
                src = PSG[:, 0 : nq * DIM]
                fd = nq * DIM
                cost = {"dve": fd * 1.042 + 160.0, "act": fd * 0.833 + 260.0}
                e = min(eng_load, key=lambda k: eng_load[k] + cost[k])
                eng_load[e] += cost[e]
                if e == "dve":
                    nc.vector.tensor_copy(dst, src)
                else:
                    nc.scalar.activation(dst, src, AF.Copy)
                ent[3] += nq
                if ent[3] == r1 - r0:
                    nc.sync.dma_start(
                        out_ap[t * 128 : (t + 1) * 128, r0 * DIM : r1 * DIM], ot[:]
                    )

            def emit_bands(si):
                c0, c1, p0, p1 = SLICES[si]
                for t in range(N_TILES):
                    for q0 in range(p0, p1, 2):
                        nq = min(2, p1 - q0)
                        PSG = psum_pool.tile([128, 1024], f32, space="PSUM",
                                             name="ps")
                        for k in range(nq):
                            b = PBANDS[q0 + k]
                            segs = PLACEMENT[b]
                            for sk, (c, slot, n, _src) in enumerate(segs):
                                sj, crel = chunk_slice[c]
                                nc.tensor.matmul(
                                    PSG[:, k * DIM : (k + 1) * DIM],
                                    XPs[sj][slot : slot + n,
                                            crel * TOK + t * 128
                                            : crel * TOK + t * 128 + 128],
                                    WGs[sj][slot : slot + n,
                                            crel * DIM : (crel + 1) * DIM],
                                    start=(sk == 0), stop=(sk == len(segs) - 1),
                                    tile_position=(slot, 0),
                                )
                        evac_quad(q0, nq, PSG, t)

            # ---- pipeline: 1-slice lookahead keeps every engine fed
            def emit_slice_norm(si):
                emit_norm(si)
                emit_prescale(si)

            emit_slice_norm(0)
            emit_slice_norm(1)
            for si in range(NSL):
                if si + 2 < NSL:
                    emit_slice_norm(si + 2)
                emit_bands(si)

    nc.compile()
    return nc


def _get_program():
    if "nc" not in _CACHE:
        _CACHE["nc"] = _build_program()
    return _CACHE["nc"]


def _prep_weights(gamma, W):
    """fold gamma + sqrt(d) + per-band int8 scale into W; compact wgc rows."""
    wf = np.asarray(W, dtype=np.float32) * np.asarray(gamma, dtype=np.float32)[:, None]
    deq = np.empty((N_BANDS,), dtype=np.float32)
    wq = np.empty_like(wf)
    for b, d in enumerate(DIM_INPUTS):
        o = OFFSETS[b]
        wb = wf[o : o + d]
        m = max(float(np.sqrt((wb * wb).sum(axis=0)).max()), 1e-30)
        s = ZQ * m / (127.0 * np.sqrt(d))
        wq[o : o + d] = wb / s
        deq[b] = s * np.sqrt(d)
    wgc = np.zeros((WGC_PAD, DIM), dtype=np.float16)
    wgc[:F_TOTAL] = wq[WGC_ROWS].astype(np.float16)
    return wgc, deq


def _host_const():
    """ssq indicator [128, IND_COLS] + expand indicator [32, 25*128]."""
    ppos = {b: p for p, b in enumerate(PBANDS)}
    slice_p0 = {}
    for (c0, c1, p0, p1) in SLICES:
        for c in range(c0, c1):
            slice_p0[c] = p0
    ind = np.zeros((128, IND_COLS), dtype=np.float16)
    indt = np.zeros((32, N_CHUNKS * 128), dtype=np.float16)
    for c, segs in enumerate(CHUNKS):
        for (slot, n, bb, soff) in segs:
            brel = ppos[bb] - slice_p0[c]
            ind[slot : slot + n, IND_OFF[c] + brel] = 1.0
            indt[brel, c * 128 + slot : c * 128 + slot + n] = 1.0
    return ind, indt


def _host_inputs(x, gamma, W):
    xf = np.ascontiguousarray(np.asarray(x, dtype=np.float32).reshape(BT, F_TOTAL))
    wgc, deq = _prep_weights(gamma, W)
    ind, indt = _host_const()

    valid = ROW_MAP >= 0
    src_rows = ROW_MAP[valid]
    in_maps = []
    for i in range(N_CORES):
        shard = xf[i * TOK : (i + 1) * TOK]
        xtp = np.zeros((F_PACK, TOK), dtype=np.float32)
        xtp[valid] = shard.T[src_rows]
        xtp = np.ascontiguousarray(
            xtp.astype(np.float16).reshape(N_CHUNKS, 128, TOK).transpose(1, 0, 2)
        ).reshape(128, N_CHUNKS * TOK)
        in_maps.append({"xt": xtp, "wgc": wgc, "ind": ind, "indt": indt})
    return in_maps, deq


def _unpack_out(packed_list, deq, b):
    bf = np.asarray(b, dtype=np.float32)
    pb = np.asarray(PBANDS)
    scale_packed = deq[pb]
    out = np.empty((BT, N_BANDS, DIM), dtype=np.float32)
    for i, pk in enumerate(packed_list):
        pk = pk.reshape(TOK, N_BANDS, DIM)
        out[i * TOK : (i + 1) * TOK, pb] = (
            pk.astype(np.float32) * scale_packed[None, :, None]
        )
    out = out.reshape(B, T, N_BANDS, DIM)
    return out + bf[None, None, :, :]


def _run(x, gamma, W, b, trace=False, trace_kwargs=None):
    nc = _get_program()
    in_maps, deq = _host_inputs(x, gamma, W)
    kw = {}
    if trace:
        kw = {"trace": True, "trace_kwargs": trace_kwargs or {}}
    res = run_bass_kernel_spmd(nc, in_maps, core_ids=list(range(N_CORES)), **kw)
    out = _unpack_out([res.results[i]["out"] for i in range(N_CORES)], deq, b)
    return out, res


def kernel(x, gamma, W, b):
    out, _ = _run(x, gamma, W, b)
    return out
